# revision 2
# baseline (speedup 1.0000x reference)
"""Causal self-attention (B=2, L=2048, E=2048, H=16, D=128) on 8 trn2 cores.

Sharding: Megatron-style tensor parallel over heads. Each core owns 2 heads:
Wq/Wk/Wv column-split, Wo row-split; x replicated (pre-transposed, bf16).
Each core computes a partial output projection; host sums partials + bias.

Device kernel per core, per (batch, head):
  - qT/kT/vT [D=128, L] built with weight slices as the stationary matmul
    operand (xT streams).  Host permutes Wq/Wk columns to (evens, odds) order
    and stacks q/k halves so RoPE is 6 contiguous [128,512] DVE ops.
  - scores per 128-row q-tile over the causal band, fp32 PSUM; diagonal block
    gets an additive -1e30 triangular mask; Exp on ACT with accum_out giving
    row sums; P normalized by reciprocal(rowsum); transposed [128,band] ->
    [128, nb, 128] with one xbar DMA-transpose; attn@v and out-projection
    accumulate in PSUM (out-proj sums the core's 2 heads).
"""

import numpy as np
import ml_dtypes

import concourse.bass as bass
import concourse.tile as tile
from concourse import bacc, mybir
from concourse.bass_utils import run_bass_kernel_spmd

BF16 = mybir.dt.bfloat16
F32 = mybir.dt.float32
AF = mybir.ActivationFunctionType
ALU = mybir.AluOpType

B, L, E = 2, 2048, 2048
H, D = 16, 128
NCORES = 8
HPC = H // NCORES          # heads per core
KT = E // 128              # 16 contraction tiles
LC = L // 512              # 4 column chunks of L
QT = L // 128              # 16 q tiles
THETA = 10000.0

_PROG = None


def _build_program():
    nc = bacc.Bacc("TRN2", target_bir_lowering=False, debug=False,
                   enable_asserts=False)

    xT_d = nc.dram_tensor("xT", [B, E, L], BF16, kind="ExternalInput").ap()
    w_d = nc.dram_tensor("w", [E, HPC * 3 * 128], BF16, kind="ExternalInput").ap()
    wo_d = nc.dram_tensor("wo", [HPC, D, E], BF16, kind="ExternalInput").ap()
    cos_d = nc.dram_tensor("cosf", [128, L], F32, kind="ExternalInput").ap()
    sin_d = nc.dram_tensor("sinf", [128, L], F32, kind="ExternalInput").ap()
    tri_d = nc.dram_tensor("tri", [128, 128], F32, kind="ExternalInput").ap()
    y_d = nc.dram_tensor("y", [B, L, E], F32, kind="ExternalOutput").ap()

    with tile.TileContext(nc) as tc:
        with tc.tile_pool(name="consts", bufs=1) as cpool, \
             tc.tile_pool(name="xt", bufs=8) as xpool, \
             tc.tile_pool(name="rope", bufs=8) as rpool, \
             tc.tile_pool(name="qkv", bufs=2) as qkvpool, \
             tc.tile_pool(name="pp", bufs=3) as ppool, \
             tc.tile_pool(name="small", bufs=8) as spool, \
             tc.tile_pool(name="outp", bufs=3) as opool, \
             tc.tile_pool(name="ys", bufs=2) as ypool, \
             tc.tile_pool(name="ps512", bufs=6, space="PSUM") as ps512, \
             tc.tile_pool(name="psy", bufs=2, space="PSUM") as psy:

            w_sb = cpool.tile([128, KT, HPC, 3, 128], BF16, tag="w")
            nc.sync.dma_start(w_sb[:], w_d.rearrange("(kt p) c -> p kt c", p=128))
            wo_sb = cpool.tile([128, HPC, E], BF16, tag="wo")
            nc.sync.dma_start(wo_sb[:], wo_d.rearrange("h p e -> p h e"))
            cosf = cpool.tile([128, L], F32, tag="cos")
            nc.sync.dma_start(cosf[:], cos_d[:])
            sinf = cpool.tile([128, L], F32, tag="sin")
            nc.sync.dma_start(sinf[:], sin_d[:])
            tri = cpool.tile([128, 128], F32, tag="tri")
            nc.sync.dma_start(tri[:], tri_d[:])

            for b in range(B):
                outT = [None] * HPC
                for h in range(HPC):
                    # ---- QKV projection + RoPE ----
                    qT = qkvpool.tile([128, L], BF16, tag="qT")
                    kTt = qkvpool.tile([128, L], BF16, tag="kT")
                    vTs = qkvpool.tile([128, L], BF16, tag="vT")
                    for lc in range(LC):
                        ls = lc * 512
                        pA = ps512.tile([128, 512], F32, tag="ps512")
                        pB = ps512.tile([128, 512], F32, tag="ps512")
                        pV = ps512.tile([128, 512], F32, tag="ps512")
                        for kt in range(KT):
                            xt = xpool.tile([128, 512], BF16, tag="xt")
                            nc.sync.dma_start(
                                xt[:], xT_d[b, kt * 128:(kt + 1) * 128, ls:ls + 512])
                            st = kt == 0
                            sp = kt == KT - 1
                            nc.tensor.matmul(pA[:], w_sb[:, kt, h, 0, :], xt[:],
                                             start=st, stop=sp)
                            nc.tensor.matmul(pB[:], w_sb[:, kt, h, 1, :], xt[:],
                                             start=st, stop=sp)
                            nc.tensor.matmul(pV[:], w_sb[:, kt, h, 2, :], xt[:],
                                             start=st, stop=sp)
                        # RoPE: A = [x1q;x1k], B = [x2q;x2k]
                        t1 = rpool.tile([128, 512], F32, tag="rt")
                        nc.vector.tensor_mul(t1[:], pA[:], cosf[:, ls:ls + 512])
                        t2 = rpool.tile([128, 512], F32, tag="rt")
                        nc.vector.tensor_mul(t2[:], pB[:], sinf[:, ls:ls + 512])
                        et = rpool.tile([128, 512], BF16, tag="ro")
                        nc.vector.tensor_sub(et[:], t1[:], t2[:])
                        t3 = rpool.tile([128, 512], F32, tag="rt")
                        nc.vector.tensor_mul(t3[:], pA[:], sinf[:, ls:ls + 512])
                        t4 = rpool.tile([128, 512], F32, tag="rt")
                        nc.vector.tensor_mul(t4[:], pB[:], cosf[:, ls:ls + 512])
                        ot = rpool.tile([128, 512], BF16, tag="ro")
                        nc.vector.tensor_add(ot[:], t3[:], t4[:])
                        nc.vector.tensor_copy(qT[0:64, ls:ls + 512], et[0:64, :])
                        nc.vector.tensor_copy(qT[64:128, ls:ls + 512], ot[0:64, :])
                        nc.vector.tensor_copy(kTt[0:64, ls:ls + 512], et[64:128, :])
                        nc.vector.tensor_copy(kTt[64:128, ls:ls + 512], ot[64:128, :])
                        nc.scalar.copy(vTs[:, ls:ls + 512], pV[:])
                    v_nat = qkvpool.tile([128, KT, 128], BF16, tag="vn")
                    nc.sync.dma_start_transpose(out=v_nat[:], in_=vTs[:])

                    # ---- attention ----
                    outT[h] = opool.tile([128, L], BF16, tag="outT",
                                         name=f"outT_b{b}h{h}")
                    po = None
                    for i in range(QT):
                        band = (i + 1) * 128
                        nch = (band + 511) // 512
                        pt_t = ppool.tile([128, L], BF16, tag="P")
                        ds = spool.tile([128, 4], F32, tag="ds")
                        for c in range(nch):
                            c0 = c * 512
                            w = min(512, band - c0)
                            s_ps = ps512.tile([128, 512], F32, tag="ps512")
                            nc.tensor.matmul(
                                s_ps[:, 0:w], qT[:, i * 128:(i + 1) * 128],
                                kTt[:, c0:c0 + w], start=True, stop=True)
                            if c == nch - 1:
                                nc.vector.tensor_tensor(
                                    s_ps[:, w - 128:w], s_ps[:, w - 128:w],
                                    tri[:], op=ALU.add)
                            nc.scalar.activation(
                                pt_t[:, c0:c0 + w], s_ps[:, 0:w], AF.Exp,
                                accum_out=ds[:, c:c + 1])
                        dtot = spool.tile([128, 1], F32, tag="dt")
                        if nch > 1:
                            nc.vector.reduce_sum(dtot[:], ds[:, 0:nch],
                                                 axis=mybir.AxisListType.X)
                        else:
                            nc.vector.tensor_copy(dtot[:], ds[:, 0:1])
                        rinv = spool.tile([128, 1], F32, tag="ri")
                        nc.vector.reciprocal(rinv[:], dtot[:])
                        nc.vector.tensor_scalar_mul(pt_t[:, 0:band], pt_t[:, 0:band],
                                                    rinv[:])
                        ptr = ppool.tile([128, KT, 128], BF16, tag="PT")
                        nc.sync.dma_start_transpose(out=ptr[:, 0:i + 1, :],
                                                    in_=pt_t[:, 0:band])
                        if i % 4 == 0:
                            po = ps512.tile([128, 512], F32, tag="ps512")
                        osl = (i % 4) * 128
                        for kb in range(i + 1):
                            nc.tensor.matmul(
                                po[:, osl:osl + 128], v_nat[:, kb, :],
                                ptr[:, kb, :], start=(kb == 0), stop=(kb == i))
                        if i % 4 == 3:
                            g0 = (i // 4) * 512
                            nc.scalar.copy(outT[h][:, g0:g0 + 512], po[:])

                # ---- output projection (sums the core's heads) ----
                for i in range(QT):
                    ysb = ypool.tile([128, E], F32, tag="ysb")
                    for ec in range(4):
                        es = ec * 512
                        yp = psy.tile([128, 512], F32, tag="psy")
                        for h in range(HPC):
                            nc.tensor.matmul(
                                yp[:], outT[h][:, i * 128:(i + 1) * 128],
                                wo_sb[:, h, es:es + 512],
                                start=(h == 0), stop=(h == HPC - 1))
                        if ec % 2 == 0:
                            nc.scalar.copy(ysb[:, es:es + 512], yp[:])
                        else:
                            nc.vector.tensor_copy(ysb[:, es:es + 512], yp[:])
                    nc.sync.dma_start(y_d[b, i * 128:(i + 1) * 128, :], ysb[:])

    nc.compile()
    return nc


def _get_program():
    global _PROG
    if _PROG is None:
        _PROG = _build_program()
    return _PROG


def make_in_maps(x, Wq, Wk, Wv, Wo):
    """Host-side sharding/layout prep. Returns list of 8 per-core input maps."""
    bf = ml_dtypes.bfloat16
    x = np.asarray(x, np.float32)
    Wq = np.asarray(Wq, np.float32)
    Wk = np.asarray(Wk, np.float32)
    Wv = np.asarray(Wv, np.float32)
    Wo = np.asarray(Wo, np.float32)

    xT = np.ascontiguousarray(x.transpose(0, 2, 1)).astype(bf)  # [B, E, L]

    inv = THETA ** (-np.arange(0, D, 2, dtype=np.float32) / D)  # [64]
    ang = np.arange(L, dtype=np.float32)[:, None] * inv[None, :]  # [L, 64]
    cosf = np.ascontiguousarray(np.concatenate([np.cos(ang).T] * 2, axis=0)
                                ).astype(np.float32)  # [128, L]
    sinf = np.ascontiguousarray(np.concatenate([np.sin(ang).T] * 2, axis=0)
                                ).astype(np.float32)
    r = np.arange(128)
    tri = np.where(r[None, :] <= r[:, None], 0.0, -1e30).astype(np.float32)

    qscale = np.float32(D ** -0.5)
    ev = np.arange(0, D, 2)
    od = np.arange(1, D, 2)

    maps = []
    for core in range(NCORES):
        w_all = np.empty((E, HPC, 3, 128), np.float32)
        for h in range(HPC):
            g = core * HPC + h
            c0 = g * D
            w_all[:, h, 0, 0:64] = Wq[:, c0 + ev] * qscale
            w_all[:, h, 0, 64:128] = Wk[:, c0 + ev]
            w_all[:, h, 1, 0:64] = Wq[:, c0 + od] * qscale
            w_all[:, h, 1, 64:128] = Wk[:, c0 + od]
            w_all[:, h, 2, :] = Wv[:, c0:c0 + D]
        wo_c = Wo[core * HPC * D:(core + 1) * HPC * D, :].reshape(HPC, D, E)
        maps.append({
            "xT": xT,
            "w": np.ascontiguousarray(w_all.reshape(E, HPC * 3 * 128)).astype(bf),
            "wo": np.ascontiguousarray(wo_c).astype(bf),
            "cosf": cosf,
            "sinf": sinf,
            "tri": tri,
        })
    return maps


def kernel(x, Wq, Wk, Wv, Wo, bo):
    nc = _get_program()
    maps = make_in_maps(x, Wq, Wk, Wv, Wo)
    res = run_bass_kernel_spmd(nc, maps, core_ids=list(range(NCORES)))
    y = np.zeros((B, L, E), np.float64)
    for c in range(NCORES):
        y += np.asarray(res.results[c]["y"], np.float64)
    y += np.asarray(bo, np.float64)[None, None, :]
    return y.astype(np.float32)


# revision 13
# speedup vs baseline: 1.4337x; 1.4337x over previous
"""Causal self-attention (B=2, L=2048, E=2048, H=16, D=128) on 8 trn2 cores.

Sharding: Megatron-style tensor parallel over heads. Each core owns 2 heads:
Wq/Wk/Wv column-split, Wo row-split; x replicated (pre-transposed, bf16).
Each core computes a partial output projection; host sums partials + bias.

Device kernel per core, per (batch, head):
  - qT/kT/vT [D=128, L] built with weight slices as the stationary matmul
    operand (xT streams).  Host permutes Wq/Wk columns to (evens, odds) order
    and stacks q/k halves so RoPE is 6 contiguous [128,512] DVE ops.
  - scores per 128-row q-tile over the causal band, fp32 PSUM; diagonal block
    gets an additive -1e30 triangular mask; Exp on ACT with accum_out giving
    row sums; P normalized by reciprocal(rowsum); transposed [128,band] ->
    [128, nb, 128] with one xbar DMA-transpose; attn@v and out-projection
    accumulate in PSUM (out-proj sums the core's 2 heads).
"""

import os

import numpy as np
import ml_dtypes

import concourse.bass as bass
import concourse.tile as tile
from concourse import bacc, mybir
from concourse.bass_utils import run_bass_kernel_spmd

BF16 = mybir.dt.bfloat16
F32 = mybir.dt.float32
AF = mybir.ActivationFunctionType
ALU = mybir.AluOpType

B, L, E = 2, 2048, 2048
H, D = 16, 128
NCORES = 8
HPC = H // NCORES          # heads per core
KT = E // 128              # 16 contraction tiles
LC = L // 512              # 4 column chunks of L
QT = L // 128              # 16 q tiles
THETA = 10000.0

_PROG = None


def _build_program():
    nc = bacc.Bacc("TRN2", target_bir_lowering=False, debug=False,
                   enable_asserts=False)

    xT_d = nc.dram_tensor("xT", [B, E, L], BF16, kind="ExternalInput").ap()
    w_d = nc.dram_tensor("w", [E, HPC * 3 * 128], BF16, kind="ExternalInput").ap()
    wo_d = nc.dram_tensor("wo", [HPC, D, E], BF16, kind="ExternalInput").ap()
    cos_d = nc.dram_tensor("cosf", [128, L], F32, kind="ExternalInput").ap()
    sin_d = nc.dram_tensor("sinf", [128, L], F32, kind="ExternalInput").ap()
    tri_d = nc.dram_tensor("tri", [128, 128], F32, kind="ExternalInput").ap()
    y_d = nc.dram_tensor("y", [B, L, E], BF16, kind="ExternalOutput").ap()

    with tile.TileContext(nc) as tc:
        with tc.tile_pool(name="consts", bufs=1) as cpool, \
             tc.tile_pool(name="xt", bufs=8) as xpool, \
             tc.tile_pool(name="rope", bufs=8) as rpool, \
             tc.tile_pool(name="qkv", bufs=2) as qkvpool, \
             tc.tile_pool(name="pp", bufs=3) as ppool, \
             tc.tile_pool(name="small", bufs=8) as spool, \
             tc.tile_pool(name="outp", bufs=12) as opool, \
             tc.tile_pool(name="ys", bufs=2) as ypool, \
             tc.tile_pool(name="ps512", bufs=6, space="PSUM") as ps512, \
             tc.tile_pool(name="psy", bufs=2, space="PSUM") as psy:

            w_sb = cpool.tile([128, KT, HPC, 3, 128], BF16, tag="w")
            nc.sync.dma_start(w_sb[:], w_d.rearrange("(kt p) c -> p kt c", p=128))
            wo_sb = cpool.tile([128, HPC, E], BF16, tag="wo")
            nc.sync.dma_start(wo_sb[:], wo_d.rearrange("h p e -> p h e"))
            cosf = cpool.tile([128, L], F32, tag="cos")
            nc.sync.dma_start(cosf[:], cos_d[:])
            sinf = cpool.tile([128, L], F32, tag="sin")
            nc.sync.dma_start(sinf[:], sin_d[:])
            tri = cpool.tile([128, 128], F32, tag="tri")
            nc.sync.dma_start(tri[:], tri_d[:])

            for rep in range(int(os.environ.get("KREP", "1"))):
              for b in range(B):
                outT = [None] * HPC
                for h in range(HPC):
                    # ---- QKV projection + RoPE ----
                    qT = qkvpool.tile([128, L], BF16, tag="qT")
                    kTt = qkvpool.tile([128, L], BF16, tag="kT")
                    vTs = qkvpool.tile([128, L], BF16, tag="vT")
                    for lc in range(LC):
                        ls = lc * 512
                        pA = ps512.tile([128, 512], F32, tag="ps512")
                        pB = ps512.tile([128, 512], F32, tag="ps512")
                        pV = ps512.tile([128, 512], F32, tag="ps512")
                        for kt in range(KT):
                            xt = xpool.tile([128, 512], BF16, tag="xt")
                            nc.sync.dma_start(
                                xt[:], xT_d[b, kt * 128:(kt + 1) * 128, ls:ls + 512])
                            st = kt == 0
                            sp = kt == KT - 1
                            nc.tensor.matmul(pA[:], w_sb[:, kt, h, 0, :], xt[:],
                                             start=st, stop=sp)
                            nc.tensor.matmul(pB[:], w_sb[:, kt, h, 1, :], xt[:],
                                             start=st, stop=sp)
                            nc.tensor.matmul(pV[:], w_sb[:, kt, h, 2, :], xt[:],
                                             start=st, stop=sp)
                        # RoPE: A = [x1q;x1k], B = [x2q;x2k]
                        t1 = rpool.tile([128, 512], F32, tag="rt")
                        nc.vector.tensor_mul(t1[:], pA[:], cosf[:, ls:ls + 512])
                        t2 = rpool.tile([128, 512], F32, tag="rt")
                        nc.vector.tensor_mul(t2[:], pB[:], sinf[:, ls:ls + 512])
                        et = rpool.tile([128, 512], BF16, tag="ro")
                        nc.gpsimd.tensor_sub(et[:], t1[:], t2[:])
                        t3 = rpool.tile([128, 512], F32, tag="rt")
                        nc.vector.tensor_mul(t3[:], pA[:], sinf[:, ls:ls + 512])
                        t4 = rpool.tile([128, 512], F32, tag="rt")
                        nc.vector.tensor_mul(t4[:], pB[:], cosf[:, ls:ls + 512])
                        ot = rpool.tile([128, 512], BF16, tag="ro")
                        nc.gpsimd.tensor_add(ot[:], t3[:], t4[:])
                        nc.vector.tensor_copy(qT[0:64, ls:ls + 512], et[0:64, :])
                        nc.vector.tensor_copy(qT[64:128, ls:ls + 512], ot[0:64, :])
                        nc.vector.tensor_copy(kTt[0:64, ls:ls + 512], et[64:128, :])
                        nc.vector.tensor_copy(kTt[64:128, ls:ls + 512], ot[64:128, :])
                        nc.scalar.copy(vTs[:, ls:ls + 512], pV[:])
                    v_nat = qkvpool.tile([128, KT, 128], BF16, tag="vn")
                    nc.scalar.dma_start_transpose(out=v_nat[:], in_=vTs[:])

                    # ---- attention ----
                    outT[h] = [
                        opool.tile([128, 512], BF16, tag="outT",
                                   name=f"outT_b{b}h{h}g{g}")
                        for g in range(4)
                    ]
                    po = None
                    for i in range(QT):
                        band = (i + 1) * 128
                        nch = (band + 511) // 512
                        pt_t = ppool.tile([128, L], BF16, tag="P")
                        ds = spool.tile([128, 4], F32, tag="ds")
                        for c in range(nch):
                            c0 = c * 512
                            w = min(512, band - c0)
                            s_ps = ps512.tile([128, 512], F32, tag="ps512")
                            nc.tensor.matmul(
                                s_ps[:, 0:w], qT[:, i * 128:(i + 1) * 128],
                                kTt[:, c0:c0 + w], start=True, stop=True)
                            if c == nch - 1:
                                nc.vector.tensor_tensor(
                                    s_ps[:, w - 128:w], s_ps[:, w - 128:w],
                                    tri[:], op=ALU.add)
                            nc.scalar.activation(
                                pt_t[:, c0:c0 + w], s_ps[:, 0:w], AF.Exp,
                                accum_out=ds[:, c:c + 1])
                        dtot = spool.tile([128, 1], F32, tag="dt")
                        if nch > 1:
                            nc.vector.reduce_sum(dtot[:], ds[:, 0:nch],
                                                 axis=mybir.AxisListType.X)
                        else:
                            nc.vector.tensor_copy(dtot[:], ds[:, 0:1])
                        rinv = spool.tile([128, 1], F32, tag="ri")
                        nc.vector.reciprocal(rinv[:], dtot[:])
                        nc.vector.tensor_scalar_mul(pt_t[:, 0:band], pt_t[:, 0:band],
                                                    rinv[:])
                        ptr = ppool.tile([128, KT, 128], BF16, tag="PT")
                        nc.scalar.dma_start_transpose(out=ptr[:, 0:i + 1, :],
                                                      in_=pt_t[:, 0:band])
                        if i % 4 == 0:
                            po = ps512.tile([128, 512], F32, tag="ps512")
                        osl = (i % 4) * 128
                        for kb in range(i + 1):
                            nc.tensor.matmul(
                                po[:, osl:osl + 128], v_nat[:, kb, :],
                                ptr[:, kb, :], start=(kb == 0), stop=(kb == i))
                        if i % 4 == 3:
                            nc.scalar.copy(outT[h][i // 4][:], po[:])

                # ---- output projection (sums the core's heads) ----
                for i in range(QT):
                    ysb = ypool.tile([128, E], BF16, tag="ysb")
                    qs = (i % 4) * 128
                    for ec in range(4):
                        es = ec * 512
                        yp = psy.tile([128, 512], F32, tag="psy")
                        for h in range(HPC):
                            nc.tensor.matmul(
                                yp[:], outT[h][i // 4][:, qs:qs + 128],
                                wo_sb[:, h, es:es + 512],
                                start=(h == 0), stop=(h == HPC - 1))
                        if ec % 2 == 0:
                            nc.scalar.copy(ysb[:, es:es + 512], yp[:])
                        else:
                            nc.vector.tensor_copy(ysb[:, es:es + 512], yp[:])
                    nc.scalar.dma_start(y_d[b, i * 128:(i + 1) * 128, :], ysb[:])

    nc.compile()
    return nc


def _get_program():
    global _PROG
    if _PROG is None:
        _PROG = _build_program()
    return _PROG


def make_in_maps(x, Wq, Wk, Wv, Wo):
    """Host-side sharding/layout prep. Returns list of 8 per-core input maps."""
    bf = ml_dtypes.bfloat16
    x = np.asarray(x, np.float32)
    Wq = np.asarray(Wq, np.float32)
    Wk = np.asarray(Wk, np.float32)
    Wv = np.asarray(Wv, np.float32)
    Wo = np.asarray(Wo, np.float32)

    xT = np.ascontiguousarray(x.transpose(0, 2, 1)).astype(bf)  # [B, E, L]

    inv = THETA ** (-np.arange(0, D, 2, dtype=np.float32) / D)  # [64]
    ang = np.arange(L, dtype=np.float32)[:, None] * inv[None, :]  # [L, 64]
    cosf = np.ascontiguousarray(np.concatenate([np.cos(ang).T] * 2, axis=0)
                                ).astype(np.float32)  # [128, L]
    sinf = np.ascontiguousarray(np.concatenate([np.sin(ang).T] * 2, axis=0)
                                ).astype(np.float32)
    r = np.arange(128)
    tri = np.where(r[None, :] <= r[:, None], 0.0, -1e30).astype(np.float32)

    qscale = np.float32(D ** -0.5)
    ev = np.arange(0, D, 2)
    od = np.arange(1, D, 2)

    maps = []
    for core in range(NCORES):
        w_all = np.empty((E, HPC, 3, 128), np.float32)
        for h in range(HPC):
            g = core * HPC + h
            c0 = g * D
            w_all[:, h, 0, 0:64] = Wq[:, c0 + ev] * qscale
            w_all[:, h, 0, 64:128] = Wk[:, c0 + ev]
            w_all[:, h, 1, 0:64] = Wq[:, c0 + od] * qscale
            w_all[:, h, 1, 64:128] = Wk[:, c0 + od]
            w_all[:, h, 2, :] = Wv[:, c0:c0 + D]
        wo_c = Wo[core * HPC * D:(core + 1) * HPC * D, :].reshape(HPC, D, E)
        maps.append({
            "xT": xT,
            "w": np.ascontiguousarray(w_all.reshape(E, HPC * 3 * 128)).astype(bf),
            "wo": np.ascontiguousarray(wo_c).astype(bf),
            "cosf": cosf,
            "sinf": sinf,
            "tri": tri,
        })
    return maps


def kernel(x, Wq, Wk, Wv, Wo, bo):
    nc = _get_program()
    maps = make_in_maps(x, Wq, Wk, Wv, Wo)
    res = run_bass_kernel_spmd(nc, maps, core_ids=list(range(NCORES)))
    y = np.zeros((B, L, E), np.float64)
    for c in range(NCORES):
        y += np.asarray(res.results[c]["y"], np.float64)
    y += np.asarray(bo, np.float64)[None, None, :]
    return y.astype(np.float32)
